# revision 1
# baseline (speedup 1.0000x reference)
"""MoE kernel for TRN2, 8 NeuronCores, data-parallel over the batch dim.

Reference computation (B=8192, D=1024, H=1024, E=16):
    weights = softmax(x @ Wg + bg, axis=1)            # [B, E]
    h       = relu(einsum('bd,edh->beh', x, W1) + b1) # [B, E, H]
    eo      = einsum('beh,eh->be', h, W2) + b2        # [B, E]
    out     = sum(eo * weights, axis=1, keepdims=True)# [B, 1]

Strategy:
  - Shard B over 8 cores (1024 rows/core); weights replicated.
  - All matmuls contract over the partition dim, so x is transposed on the
    HOST (free) and each core gets xT [D, 1024] resident in SBUF.
  - Stage 1 (per (e, h_tile)=t of 128): psum[h=128, b=512x2] accumulated
    over 8 d-tiles; float32r matmuls (full PE rate at N>=256).
  - ReLU+b1 via ScalarE activation (bias is per-partition in h-major layout).
  - Stage 2: W2 built block-diagonal on host -> every t contributes one
    [128hx16e] @ [128h, 512b] matmul accumulating into ONE [16, 1024] psum
    tile; all 16 experts' outputs land stacked on partitions 0..15.
  - Gating: fp32 matmuls into [128b, 16e] psum; softmax along the free dim;
    exp(bg) folded in multiplicatively (softmax is shift/scale invariant).
  - Combine: eoT + b2 -> PE-transpose 16x128 chunks -> [128b, 16e]; multiply
    by gate weights, reduce along free dim -> [128, 1] -> DMA out.
"""

import numpy as np

import concourse.bacc as bacc
import concourse.bass as bass
import concourse.mybir as mybir
from concourse import tile
from concourse.bass_utils import run_bass_kernel_spmd

B, D, H, E = 8192, 1024, 1024, 16
N_CORES = 8
BS = B // N_CORES  # 1024 batch rows per core
NB = BS // 128     # 8 b-tiles of 128
BH = 512           # half-batch moving-operand width (one psum bank)
DT = D // 128      # 8 d-tiles
HT = H // 128      # 8 h-tiles
T = E * HT         # 128 (e, h_tile) pairs

F32 = mybir.dt.float32
F32R = mybir.dt.float32r
AF = mybir.ActivationFunctionType
AX = mybir.AxisListType


def build_bass():
    nc = bacc.Bacc("TRN2", target_bir_lowering=False, debug=False)
    xt_d = nc.dram_tensor("xt", [D, BS], F32R, kind="ExternalInput")
    w1_d = nc.dram_tensor("w1p", [T, 128, DT * 128], F32R, kind="ExternalInput")
    b1t_d = nc.dram_tensor("b1t", [128, T], F32, kind="ExternalInput")
    w2bd_d = nc.dram_tensor("w2bd", [128, T * E], F32R, kind="ExternalInput")
    wgp_d = nc.dram_tensor("wgp", [128, DT * E], F32R, kind="ExternalInput")
    ebg_d = nc.dram_tensor("ebg", [128, E], F32, kind="ExternalInput")
    b2_d = nc.dram_tensor("b2p", [E, 1], F32, kind="ExternalInput")
    id16_d = nc.dram_tensor("id16", [E, E], F32, kind="ExternalInput")
    y_d = nc.dram_tensor("y", [BS, 1], F32, kind="ExternalOutput")

    with tile.TileContext(nc) as tc:
        with (
            tc.tile_pool(name="const", bufs=1) as cpool,
            tc.tile_pool(name="w1", bufs=4) as w1pool,
            tc.tile_pool(name="hrelu", bufs=4) as hpool,
            tc.tile_pool(name="sm", bufs=2) as smpool,
            tc.tile_pool(name="ps_h", bufs=2, space=bass.MemorySpace.PSUM) as psh,
            tc.tile_pool(name="ps_eo", bufs=1, space=bass.MemorySpace.PSUM) as pseo,
            tc.tile_pool(name="ps_s", bufs=2, space=bass.MemorySpace.PSUM) as pss,
        ):
            # ---- resident tensors ----
            xt_sb = []
            for d in range(DT):
                tl = cpool.tile([128, BS], F32R, tag=f"xt{d}")
                nc.sync.dma_start(tl[:], xt_d[d * 128:(d + 1) * 128, :])
                xt_sb.append(tl)
            w2bd_sb = cpool.tile([128, T * E], F32R, tag="w2bd")
            nc.sync.dma_start(w2bd_sb[:], w2bd_d[:])
            b1t_sb = cpool.tile([128, T], F32, tag="b1t")
            nc.sync.dma_start(b1t_sb[:], b1t_d[:])
            wgp_sb = cpool.tile([128, DT * E], F32R, tag="wgp")
            nc.sync.dma_start(wgp_sb[:], wgp_d[:])
            ebg_sb = cpool.tile([128, E], F32, tag="ebg")
            nc.sync.dma_start(ebg_sb[:], ebg_d[:])
            b2_sb = cpool.tile([E, 1], F32, tag="b2")
            nc.sync.dma_start(b2_sb[:], b2_d[:])
            id16_sb = cpool.tile([E, E], F32, tag="id16")
            nc.sync.dma_start(id16_sb[:], id16_d[:])
            w_all = cpool.tile([128, NB, E], F32, tag="wall")  # gate weights
            eo_sb = cpool.tile([E, BS], F32, tag="eo")         # expert outs ^T

            # ---- gating: logits -> softmax along free dim ----
            for bt in range(NB):
                ps_g = pss.tile([128, E], F32, tag="sps")
                for d in range(DT):
                    nc.tensor.matmul(
                        ps_g[:],
                        xt_sb[d][:, bt * 128:(bt + 1) * 128],
                        wgp_sb[:, d * E:(d + 1) * E],
                        start=(d == 0), stop=(d == DT - 1),
                        skip_group_check=True,
                    )
                pexp = smpool.tile([128, E], F32, tag="pexp")
                nc.scalar.activation(pexp[:], ps_g[:], AF.Exp)
                nc.vector.tensor_mul(pexp[:], pexp[:], ebg_sb[:])
                ssum = smpool.tile([128, 1], F32, tag="ssum")
                nc.vector.reduce_sum(ssum[:], pexp[:], axis=AX.X)
                rsum = smpool.tile([128, 1], F32, tag="rsum")
                nc.vector.reciprocal(rsum[:], ssum[:])
                nc.vector.tensor_scalar_mul(w_all[:, bt, :], pexp[:], rsum[:])

            # ---- main loop over t=(e, h_tile) ----
            eo_ps = pseo.tile([E, BS], F32)

            def emit_stage2(t, hr):
                for bh in range(2):
                    nc.tensor.matmul(
                        eo_ps[:, bh * BH:(bh + 1) * BH],
                        w2bd_sb[:, t * E:(t + 1) * E],
                        hr[:, bh * BH:(bh + 1) * BH],
                        start=(t == 0), stop=(t == T - 1),
                        skip_group_check=True,
                    )

            pending = []
            for t in range(T):
                w1t = w1pool.tile([128, DT * 128], F32R, tag="w1t")
                nc.sync.dma_start(w1t[:], w1_d[t, :, :])
                ps1 = psh.tile([128, BS], F32, tag="ps1")
                for d in range(DT):
                    lhs = w1t[:, d * 128:(d + 1) * 128]
                    for bh in range(2):
                        nc.tensor.matmul(
                            ps1[:, bh * BH:(bh + 1) * BH],
                            lhs,
                            xt_sb[d][:, bh * BH:(bh + 1) * BH],
                            start=(d == 0), stop=(d == DT - 1),
                            skip_group_check=True,
                        )
                if pending:
                    emit_stage2(*pending.pop())
                hr = hpool.tile([128, BS], F32R, tag="hr")
                for bh in range(2):
                    nc.scalar.activation(
                        hr[:, bh * BH:(bh + 1) * BH],
                        ps1[:, bh * BH:(bh + 1) * BH],
                        AF.Relu,
                        bias=b1t_sb[:, t:t + 1],
                    )
                pending.append((t, hr))
            emit_stage2(*pending.pop())

            # ---- combine: (eoT + b2) -> transpose -> * gates -> reduce ----
            nc.vector.tensor_scalar_add(eo_sb[:], eo_ps[:], b2_sb[:])
            for bt in range(NB):
                tps = pss.tile([128, E], F32, tag="sps")
                nc.tensor.transpose(
                    tps[:], eo_sb[:, bt * 128:(bt + 1) * 128], id16_sb[:]
                )
                eo_bt = smpool.tile([128, E], F32, tag="eobt")
                nc.vector.tensor_copy(eo_bt[:], tps[:])
                prod = smpool.tile([128, E], F32, tag="prod")
                nc.vector.tensor_mul(prod[:], eo_bt[:], w_all[:, bt, :])
                y_t = smpool.tile([128, 1], F32, tag="yt")
                nc.vector.reduce_sum(y_t[:], prod[:], axis=AX.X)
                nc.sync.dma_start(y_d[bt * 128:(bt + 1) * 128, :], y_t[:])
    nc.compile()
    return nc


def round_fp32r(a):
    """Round fp32 to the FP32R format: 11-bit mantissa, RNE, low 12 bits 0."""
    u = np.ascontiguousarray(a, dtype=np.float32).view(np.uint32)
    lsb = (u >> np.uint32(12)) & np.uint32(1)
    r = (u + np.uint32(0x7FF) + lsb) & np.uint32(0xFFFFF000)
    return r.view(np.float32)


def prep_inputs(x, W1, b1, W2, b2, Wg, bg):
    """Host-side data prep. Returns (shared_map, per_core_xt)."""
    f = np.float32
    # W1 [E, D, H] -> [t=(e,ht), d_in, (d_t, h_in)] so each t is one
    # contiguous 512KB block whose SBUF layout is [128 d_in, 8 d_t * 128 h]
    w1p = np.ascontiguousarray(
        W1.reshape(E, DT, 128, HT, 128).transpose(0, 3, 2, 1, 4)
        .reshape(T, 128, DT * 128).astype(f))
    w1p = round_fp32r(w1p)
    b1t = np.ascontiguousarray(
        b1.reshape(E, HT, 128).transpose(2, 0, 1).reshape(128, T).astype(f))
    w2bd = np.zeros((128, T, E), dtype=f)
    for t in range(T):
        e, ht = divmod(t, HT)
        w2bd[:, t, e] = W2[e, ht * 128:(ht + 1) * 128]
    w2bd = round_fp32r(w2bd.reshape(128, T * E))
    wgp = np.ascontiguousarray(
        Wg.reshape(DT, 128, E).transpose(1, 0, 2).reshape(128, DT * E).astype(f))
    wgp = round_fp32r(wgp)
    ebg = np.broadcast_to(np.exp(bg.astype(f))[None, :], (128, E)).copy()
    b2p = np.ascontiguousarray(b2.astype(f).reshape(E, 1))
    id16 = np.eye(E, dtype=f)
    shared = {"w1p": w1p, "b1t": b1t, "w2bd": w2bd, "wgp": wgp,
              "ebg": ebg, "b2p": b2p, "id16": id16}
    xT = round_fp32r(np.ascontiguousarray(x.astype(f).T))  # [D, B]
    xts = [np.ascontiguousarray(xT[:, c * BS:(c + 1) * BS]) for c in range(N_CORES)]
    return shared, xts


def run(inputs, trace=False):
    nc = build_bass()
    shared, xts = prep_inputs(**inputs)
    in_maps = [dict(shared, xt=xts[c]) for c in range(N_CORES)]
    res = run_bass_kernel_spmd(
        nc, in_maps, core_ids=list(range(N_CORES)), trace=trace
    )
    y = np.concatenate([r["y"] for r in res.results], axis=0)
    return y, res


def kernel(**inputs):
    y, _ = run(inputs, trace=False)
    return y


if __name__ == "__main__":
    rng = np.random.default_rng(0)
    ins = {
        "x": rng.standard_normal((B, D), dtype=np.float32),
        "W1": rng.standard_normal((E, D, H), dtype=np.float32) / 32,
        "b1": rng.standard_normal((E, H), dtype=np.float32) / 32,
        "W2": rng.standard_normal((E, H), dtype=np.float32) / 32,
        "b2": rng.standard_normal((E,), dtype=np.float32) / 32,
        "Wg": rng.standard_normal((D, E), dtype=np.float32) / 32,
        "bg": rng.standard_normal((E,), dtype=np.float32) / 32,
    }
    y = kernel(**ins)
    print("ok", y.shape, y.dtype)



# revision 3
# speedup vs baseline: 1.4320x; 1.4320x over previous
"""MoE kernel for TRN2, 8 NeuronCores, data-parallel over the batch dim.

Reference computation (B=8192, D=1024, H=1024, E=16):
    weights = softmax(x @ Wg + bg, axis=1)            # [B, E]
    h       = relu(einsum('bd,edh->beh', x, W1) + b1) # [B, E, H]
    eo      = einsum('beh,eh->be', h, W2) + b2        # [B, E]
    out     = sum(eo * weights, axis=1, keepdims=True)# [B, 1]

Strategy (v2 — bf16 matmuls, col-tiled stage 2, transposed combine):
  - Shard B over 8 cores (1024 rows/core); weights replicated.
  - All heavy matmuls in bf16 (1 cycle/row on PE + fast weight load); the
    d-contraction runs in fp32 PSUM so accuracy stays ~0.3%.
  - Stage 1 per t=(e, h_tile): psum[h=128, b=512] x2 accumulated over 8
    d-tiles from resident xT tiles; ReLU+b1 via ScalarE -> hr bf16.
  - Stage 2: w2 column blocks, 4 PSUM col-groups (partitions 32j..32j+15,
    j=t%4): batches of 4 matmuls on distinct col-groups run concurrently
    in the PE array (~4x cheaper than serial); groups are summed later via
    a replicated-weight reduction.
  - Gating stays transposed end-to-end: logits^T [128, B] with gate
    weights replicated into all 4 col-groups (pad cols zero); softmax is
    exp on ScalarE (bias=bg, pad rows -40 -> 0) + one PE reduction with a
    0.25-weighted ones vector (each expert appears 4x); no transposes.
  - Combine: v = eo_psum * expw (DVE); num = ones^T @ v + (b2/4)^T @ expw;
    y = num * reciprocal(sumexp); y^T DMA'd out as a [1, B] row.
"""

import numpy as np
import ml_dtypes

import concourse.bacc as bacc
import concourse.bass as bass
import concourse.mybir as mybir
from concourse import tile
from concourse.bass_utils import run_bass_kernel_spmd

B, D, H, E = 8192, 1024, 1024, 16
N_CORES = 8
BS = B // N_CORES  # 1024 batch rows per core
BH = 512           # psum-bank-sized half of the batch
DT = D // 128      # 8 d-tiles
HT = H // 128      # 8 h-tiles
T = E * HT         # 128 (e, h_tile) pairs
GB = 4             # stage-2 col-groups

F32 = mybir.dt.float32
F32R = mybir.dt.float32r
BF16 = mybir.dt.bfloat16
AF = mybir.ActivationFunctionType


def build_bass():
    nc = bacc.Bacc("TRN2", target_bir_lowering=False, debug=False)
    xt_d = nc.dram_tensor("xt", [D, BS], BF16, kind="ExternalInput")
    w1_d = nc.dram_tensor("w1p", [T, 128, DT * 128], BF16, kind="ExternalInput")
    b1t_d = nc.dram_tensor("b1t", [128, T], F32, kind="ExternalInput")
    w2bd_d = nc.dram_tensor("w2bd", [128, T * E], BF16, kind="ExternalInput")
    wg4_d = nc.dram_tensor("wg4", [128, DT * 128], BF16, kind="ExternalInput")
    bg4_d = nc.dram_tensor("bg4", [128, 1], F32, kind="ExternalInput")
    b2q4_d = nc.dram_tensor("b2q4", [128, 1], F32R, kind="ExternalInput")
    ones1_d = nc.dram_tensor("ones1", [128, 1], F32R, kind="ExternalInput")
    o025_d = nc.dram_tensor("o025", [128, 1], F32R, kind="ExternalInput")
    y_d = nc.dram_tensor("y", [1, BS], F32, kind="ExternalOutput")

    with tile.TileContext(nc) as tc:
        with (
            tc.tile_pool(name="const", bufs=1) as cpool,
            tc.tile_pool(name="w1", bufs=4) as w1pool,
            tc.tile_pool(name="hrelu", bufs=6) as hpool,
            tc.tile_pool(name="misc", bufs=1) as mpool,
            tc.tile_pool(name="ps1", bufs=2, space=bass.MemorySpace.PSUM) as psh,
            tc.tile_pool(name="ps_eo", bufs=1, space=bass.MemorySpace.PSUM) as pseo,
            tc.tile_pool(name="ps_aux", bufs=2, space=bass.MemorySpace.PSUM) as psaux,
        ):
            # ---- resident tensors ----
            xt_sb = []
            for d in range(DT):
                tl = cpool.tile([128, BS], BF16, tag=f"xt{d}")
                nc.sync.dma_start(tl[:], xt_d[d * 128:(d + 1) * 128, :])
                xt_sb.append(tl)
            wg4_sb = cpool.tile([128, DT * 128], BF16, tag="wg4")
            nc.sync.dma_start(wg4_sb[:], wg4_d[:])
            bg4_sb = cpool.tile([128, 1], F32, tag="bg4")
            nc.sync.dma_start(bg4_sb[:], bg4_d[:])
            w2bd_sb = cpool.tile([128, T * E], BF16, tag="w2bd")
            nc.sync.dma_start(w2bd_sb[:], w2bd_d[:])
            b1t_sb = cpool.tile([128, T], F32, tag="b1t")
            nc.sync.dma_start(b1t_sb[:], b1t_d[:])
            b2q4_sb = cpool.tile([128, 1], F32R, tag="b2q4")
            nc.sync.dma_start(b2q4_sb[:], b2q4_d[:])
            ones1_sb = cpool.tile([128, 1], F32R, tag="ones1")
            nc.sync.dma_start(ones1_sb[:], ones1_d[:])
            o025_sb = cpool.tile([128, 1], F32R, tag="o025")
            nc.sync.dma_start(o025_sb[:], o025_d[:])

            expw_sb = mpool.tile([128, BS], F32R, tag="expw")
            v_sb = mpool.tile([128, BS], F32R, tag="v")
            serec_sb = mpool.tile([1, BS], F32, tag="serec")
            y_sb = mpool.tile([1, BS], F32, tag="ysb")

            # ---- gating: logits^T, replicated into the 4 col-groups ----
            glog = psaux.tile([128, BS], F32, tag="aux")
            for d in range(DT):
                for bh in range(2):
                    nc.tensor.matmul(
                        glog[:, bh * BH:(bh + 1) * BH],
                        wg4_sb[:, d * 128:(d + 1) * 128],
                        xt_sb[d][:, bh * BH:(bh + 1) * BH],
                        start=(d == 0), stop=(d == DT - 1),
                        skip_group_check=True,
                    )
            # expw = exp(logits + bg); pad rows get bias -40 -> ~0
            nc.scalar.activation(expw_sb[:], glog[:], AF.Exp, bias=bg4_sb[:])

            # ---- stage-2 accumulator; zero pad rows once ----
            eo_ps = pseo.tile([128, BS], F32)
            nc.vector.memset(eo_ps[:], 0.0)

            hrs = {}

            def flush(ts):
                for bh in range(2):
                    for tt in ts:
                        g = tt % GB
                        nc.tensor.matmul(
                            eo_ps[32 * g:32 * g + 16, bh * BH:(bh + 1) * BH],
                            w2bd_sb[:, tt * E:(tt + 1) * E],
                            hrs[tt][:, bh * BH:(bh + 1) * BH],
                            start=(tt < GB), stop=(tt >= T - GB),
                            skip_group_check=True,
                            tile_position=(0, 32 * g),
                        )
                for tt in ts:
                    del hrs[tt]

            # ---- main loop over t=(e, h_tile) ----
            for t in range(T):
                if t % GB == 0 and t > 0:
                    flush(range(t - GB, t))
                w1t = w1pool.tile([128, DT * 128], BF16, tag="w1t")
                nc.sync.dma_start(w1t[:], w1_d[t, :, :])
                hr = hpool.tile([128, BS], BF16, tag="hr")
                for bh in range(2):
                    ps1 = psh.tile([128, BH], F32, tag="ps1")
                    for d in range(DT):
                        nc.tensor.matmul(
                            ps1[:],
                            w1t[:, d * 128:(d + 1) * 128],
                            xt_sb[d][:, bh * BH:(bh + 1) * BH],
                            start=(d == 0), stop=(d == DT - 1),
                            skip_group_check=True,
                        )
                    nc.scalar.activation(
                        hr[:, bh * BH:(bh + 1) * BH], ps1[:], AF.Relu,
                        bias=b1t_sb[:, t:t + 1],
                    )
                hrs[t] = hr
                if t == 2:
                    # sum of gate weights (each expert appears 4x -> 0.25)
                    sumexp = psaux.tile([1, BS], F32, tag="aux")
                    for bh in range(2):
                        nc.tensor.matmul(
                            sumexp[:, bh * BH:(bh + 1) * BH],
                            o025_sb[:], expw_sb[:, bh * BH:(bh + 1) * BH],
                            start=True, stop=True, skip_group_check=True,
                        )
                    nc.vector.reciprocal(serec_sb[:], sumexp[:])
            flush(range(T - GB, T))

            # ---- combine: y = (1^T(eo*expw) + (b2/4)^T expw) / sumexp ----
            nc.vector.tensor_mul(v_sb[:], eo_ps[:], expw_sb[:])
            num = psaux.tile([1, BS], F32, tag="aux")
            for bh in range(2):
                nc.tensor.matmul(
                    num[:, bh * BH:(bh + 1) * BH],
                    ones1_sb[:], v_sb[:, bh * BH:(bh + 1) * BH],
                    start=True, stop=False, skip_group_check=True,
                )
                nc.tensor.matmul(
                    num[:, bh * BH:(bh + 1) * BH],
                    b2q4_sb[:], expw_sb[:, bh * BH:(bh + 1) * BH],
                    start=False, stop=True, skip_group_check=True,
                )
            nc.vector.tensor_mul(y_sb[:], num[:], serec_sb[:])
            nc.sync.dma_start(y_d[:], y_sb[:])
    nc.compile()
    return nc


def prep_inputs(x, W1, b1, W2, b2, Wg, bg):
    """Host-side data prep. Returns (shared_map, per_core_xt)."""
    f = np.float32
    bf = ml_dtypes.bfloat16
    # W1 [E, D, H] -> [t=(e,ht), d_in, (d_t, h_in)]: per t one contiguous
    # block whose SBUF layout is [128 d_in, 8 d_t * 128 h]
    w1p = np.ascontiguousarray(
        np.asarray(W1, f).reshape(E, DT, 128, HT, 128)
        .transpose(0, 3, 2, 1, 4).reshape(T, 128, DT * 128)).astype(bf)
    b1t = np.ascontiguousarray(
        np.asarray(b1, f).reshape(E, HT, 128).transpose(2, 0, 1).reshape(128, T))
    w2bd = np.zeros((128, T, E), dtype=f)
    for t in range(T):
        e, ht = divmod(t, HT)
        w2bd[:, t, e] = W2[e, ht * 128:(ht + 1) * 128]
    w2bd = w2bd.reshape(128, T * E).astype(bf)
    # gate weights replicated into the 4 col-groups (16 used + 16 pad cols)
    wgr = np.asarray(Wg, f).reshape(DT, 128, E)
    wg4 = np.zeros((DT, 128, 128), dtype=f)
    for j in range(GB):
        wg4[:, :, 32 * j:32 * j + E] = wgr
    wg4 = np.ascontiguousarray(wg4.transpose(1, 0, 2).reshape(128, DT * 128)).astype(bf)
    lane = np.arange(128) % 32
    real = lane < E
    bg4 = np.full((128, 1), -40.0, f)
    bg4[real, 0] = np.tile(np.asarray(bg, f), GB)
    b2q4 = np.zeros((128, 1), f)
    b2q4[real, 0] = np.tile(np.asarray(b2, f) / 4.0, GB)
    ones1 = np.where(real, 1.0, 0.0).astype(f).reshape(128, 1)
    o025 = np.where(real, 0.25, 0.0).astype(f).reshape(128, 1)
    shared = {"w1p": w1p, "b1t": b1t, "w2bd": w2bd, "wg4": wg4, "bg4": bg4,
              "b2q4": b2q4, "ones1": ones1, "o025": o025}
    xT = np.asarray(x, f).T.astype(bf)  # [D, B]
    xts = [np.ascontiguousarray(xT[:, c * BS:(c + 1) * BS]) for c in range(N_CORES)]
    return shared, xts


def run(inputs, trace=False):
    nc = build_bass()
    shared, xts = prep_inputs(**inputs)
    in_maps = [dict(shared, xt=xts[c]) for c in range(N_CORES)]
    res = run_bass_kernel_spmd(
        nc, in_maps, core_ids=list(range(N_CORES)), trace=trace
    )
    y = np.concatenate([r["y"] for r in res.results], axis=1)  # [1, B]
    return np.ascontiguousarray(y.reshape(B, 1).astype(np.float32)), res


def kernel(**inputs):
    y, _ = run(inputs, trace=False)
    return y


if __name__ == "__main__":
    rng = np.random.default_rng(0)
    ins = {
        "x": rng.standard_normal((B, D), dtype=np.float32),
        "W1": rng.standard_normal((E, D, H), dtype=np.float32) / 32,
        "b1": rng.standard_normal((E, H), dtype=np.float32) / 32,
        "W2": rng.standard_normal((E, H), dtype=np.float32) / 32,
        "b2": rng.standard_normal((E,), dtype=np.float32) / 32,
        "Wg": rng.standard_normal((D, E), dtype=np.float32) / 32,
        "bg": rng.standard_normal((E,), dtype=np.float32) / 32,
    }
    y = kernel(**ins)
    print("ok", y.shape, y.dtype)


# revision 7
# speedup vs baseline: 1.4984x; 1.0464x over previous
"""MoE kernel for TRN2, 8 NeuronCores, data-parallel over the batch dim.

Reference computation (B=8192, D=1024, H=1024, E=16):
    weights = softmax(x @ Wg + bg, axis=1)            # [B, E]
    h       = relu(einsum('bd,edh->beh', x, W1) + b1) # [B, E, H]
    eo      = einsum('beh,eh->be', h, W2) + b2        # [B, E]
    out     = sum(eo * weights, axis=1, keepdims=True)# [B, 1]

Strategy (v2 — bf16 matmuls, col-tiled stage 2, transposed combine):
  - Shard B over 8 cores (1024 rows/core); weights replicated.
  - All heavy matmuls in bf16 (1 cycle/row on PE + fast weight load); the
    d-contraction runs in fp32 PSUM so accuracy stays ~0.3%.
  - Stage 1 per t=(e, h_tile): psum[h=128, b=512] x2 accumulated over 8
    d-tiles from resident xT tiles; ReLU+b1 via ScalarE -> hr bf16.
  - Stage 2: w2 column blocks, 4 PSUM col-groups (partitions 32j..32j+15,
    j=t%4): batches of 4 matmuls on distinct col-groups run concurrently
    in the PE array (~4x cheaper than serial); groups are summed later via
    a replicated-weight reduction.
  - Gating stays transposed end-to-end: logits^T [128, B] with gate
    weights replicated into all 4 col-groups (pad cols zero); softmax is
    exp on ScalarE (bias=bg, pad rows -40 -> 0) + one PE reduction with a
    0.25-weighted ones vector (each expert appears 4x); no transposes.
  - Combine: v = eo_psum * expw (DVE); num = ones^T @ v + (b2/4)^T @ expw;
    y = num * reciprocal(sumexp); y^T DMA'd out as a [1, B] row.
"""

import numpy as np
import ml_dtypes

import concourse.bacc as bacc
import concourse.bass as bass
import concourse.mybir as mybir
from concourse import tile
from concourse.bass_utils import run_bass_kernel_spmd

B, D, H, E = 8192, 1024, 1024, 16
N_CORES = 8
BS = B // N_CORES  # 1024 batch rows per core
BH = 512           # psum-bank-sized half of the batch
DT = D // 128      # 8 d-tiles
HT = H // 128      # 8 h-tiles
T = E * HT         # 128 (e, h_tile) pairs
GB = 4             # stage-2 col-groups

F32 = mybir.dt.float32
F32R = mybir.dt.float32r
BF16 = mybir.dt.bfloat16
AF = mybir.ActivationFunctionType


def build_bass():
    nc = bacc.Bacc("TRN2", target_bir_lowering=False, debug=False)
    xt_d = nc.dram_tensor("xt", [D, BS], BF16, kind="ExternalInput")
    w1_d = nc.dram_tensor("w1p", [T, 128, DT * 128], BF16, kind="ExternalInput")
    b1t_d = nc.dram_tensor("b1t", [128, T], F32, kind="ExternalInput")
    w2bd_d = nc.dram_tensor("w2bd", [128, T * E], BF16, kind="ExternalInput")
    wg4_d = nc.dram_tensor("wg4", [128, DT * 128], BF16, kind="ExternalInput")
    bg4_d = nc.dram_tensor("bg4", [128, 1], F32, kind="ExternalInput")
    b2q4_d = nc.dram_tensor("b2q4", [128, 1], F32R, kind="ExternalInput")
    ones1_d = nc.dram_tensor("ones1", [128, 1], F32R, kind="ExternalInput")
    o025_d = nc.dram_tensor("o025", [128, 1], F32R, kind="ExternalInput")
    y_d = nc.dram_tensor("y", [1, BS], F32, kind="ExternalOutput")

    with tile.TileContext(nc) as tc:
        with (
            tc.tile_pool(name="const", bufs=1) as cpool,
            tc.tile_pool(name="w1", bufs=4) as w1pool,
            tc.tile_pool(name="hrelu", bufs=10) as hpool,
            tc.tile_pool(name="misc", bufs=1) as mpool,
            tc.tile_pool(name="ps1", bufs=2, space=bass.MemorySpace.PSUM) as psh,
            tc.tile_pool(name="ps_eo", bufs=1, space=bass.MemorySpace.PSUM) as pseo,
            tc.tile_pool(name="ps_aux", bufs=2, space=bass.MemorySpace.PSUM) as psaux,
        ):
            # ---- resident tensors ----
            # All DMAs share one serial queue: order = priority. Gate weights
            # first (first matmuls need them), then xt tiles, then the small
            # consts; w2bd is deferred into the loop (first used at t=8).
            wg4_sb = cpool.tile([128, DT * 128], BF16, tag="wg4")
            nc.sync.dma_start(wg4_sb[:], wg4_d[:])
            xt_sb = []
            for d in range(DT):
                tl = cpool.tile([128, BS], BF16, tag=f"xt{d}")
                nc.sync.dma_start(tl[:], xt_d[d * 128:(d + 1) * 128, :])
                xt_sb.append(tl)
            bg4_sb = cpool.tile([128, 1], F32, tag="bg4")
            nc.sync.dma_start(bg4_sb[:], bg4_d[:])
            b1t_sb = cpool.tile([128, T], F32, tag="b1t")
            nc.sync.dma_start(b1t_sb[:], b1t_d[:])
            b2q4_sb = cpool.tile([128, 1], F32R, tag="b2q4")
            nc.sync.dma_start(b2q4_sb[:], b2q4_d[:])
            ones1_sb = cpool.tile([128, 1], F32R, tag="ones1")
            nc.sync.dma_start(ones1_sb[:], ones1_d[:])
            o025_sb = cpool.tile([128, 1], F32R, tag="o025")
            nc.sync.dma_start(o025_sb[:], o025_d[:])
            w2bd_sb = cpool.tile([128, T * E], BF16, tag="w2bd")

            expw_sb = mpool.tile([128, BS], F32R, tag="expw")
            v_sb = mpool.tile([128, BS], F32R, tag="v")
            serec_sb = mpool.tile([1, BS], F32, tag="serec")
            y_sb = mpool.tile([1, BS], F32, tag="ysb")

            # ---- gating: logits^T, replicated into the 4 col-groups ----
            glog = psaux.tile([128, BS], F32, tag="aux")
            for d in range(DT):
                for bh in range(2):
                    nc.tensor.matmul(
                        glog[:, bh * BH:(bh + 1) * BH],
                        wg4_sb[:, d * 128:(d + 1) * 128],
                        xt_sb[d][:, bh * BH:(bh + 1) * BH],
                        start=(d == 0), stop=(d == DT - 1),
                        skip_group_check=True,
                    )
            # expw = exp(logits + bg); pad rows get bias -40 -> ~0
            nc.scalar.activation(expw_sb[:], glog[:], AF.Exp, bias=bg4_sb[:])

            # ---- stage-2 accumulator; zero pad rows once ----
            eo_ps = pseo.tile([128, BS], F32)
            nc.vector.memset(eo_ps[:], 0.0)

            hrs = {}

            def flush(ts):
                for bh in range(2):
                    for tt in ts:
                        g = tt % GB
                        nc.tensor.matmul(
                            eo_ps[32 * g:32 * g + 16, bh * BH:(bh + 1) * BH],
                            w2bd_sb[:, tt * E:(tt + 1) * E],
                            hrs[tt][:, bh * BH:(bh + 1) * BH],
                            start=(tt < GB), stop=(tt >= T - GB),
                            skip_group_check=True,
                            tile_position=(0, 32 * g),
                        )
                for tt in ts:
                    del hrs[tt]

            # ---- main loop over t=(e, h_tile) ----
            # Stage-2 batches are flushed two batches behind so the PE never
            # waits on a freshly produced ReLU tile.
            for t in range(T):
                if t % GB == 0 and t >= 2 * GB:
                    flush(range(t - 2 * GB, t - GB))
                w1t = w1pool.tile([128, DT * 128], BF16, tag="w1t")
                nc.sync.dma_start(w1t[:], w1_d[t, :, :])
                if t == 2:
                    nc.sync.dma_start(w2bd_sb[:], w2bd_d[:])
                hr = hpool.tile([128, BS], BF16, tag="hr")
                for bh in range(2):
                    ps1 = psh.tile([128, BH], F32, tag="ps1")
                    for d in range(DT):
                        nc.tensor.matmul(
                            ps1[:],
                            w1t[:, d * 128:(d + 1) * 128],
                            xt_sb[d][:, bh * BH:(bh + 1) * BH],
                            start=(d == 0), stop=(d == DT - 1),
                            skip_group_check=True,
                        )
                    nc.scalar.activation(
                        hr[:, bh * BH:(bh + 1) * BH], ps1[:], AF.Relu,
                        bias=b1t_sb[:, t:t + 1],
                    )
                hrs[t] = hr
                if t == 2:
                    # sum of gate weights (each expert appears 4x -> 0.25)
                    sumexp = psaux.tile([1, BS], F32, tag="aux")
                    for bh in range(2):
                        nc.tensor.matmul(
                            sumexp[:, bh * BH:(bh + 1) * BH],
                            o025_sb[:], expw_sb[:, bh * BH:(bh + 1) * BH],
                            start=True, stop=True, skip_group_check=True,
                        )
                    nc.vector.reciprocal(serec_sb[:], sumexp[:])
            flush(range(T - 2 * GB, T - GB))
            flush(range(T - GB, T))

            # ---- combine: y = (1^T(eo*expw) + (b2/4)^T expw) / sumexp ----
            num = psaux.tile([1, BS], F32, tag="aux")
            for bh in range(2):
                sl = slice(bh * BH, (bh + 1) * BH)
                nc.vector.tensor_mul(v_sb[:, sl], eo_ps[:, sl], expw_sb[:, sl])
                nc.tensor.matmul(
                    num[:, sl], ones1_sb[:], v_sb[:, sl],
                    start=True, stop=False, skip_group_check=True,
                )
                nc.tensor.matmul(
                    num[:, sl], b2q4_sb[:], expw_sb[:, sl],
                    start=False, stop=True, skip_group_check=True,
                )
            nc.vector.tensor_mul(y_sb[:], num[:], serec_sb[:])
            nc.sync.dma_start(y_d[:], y_sb[:])
    nc.compile()
    return nc


def prep_inputs(x, W1, b1, W2, b2, Wg, bg):
    """Host-side data prep. Returns (shared_map, per_core_xt)."""
    f = np.float32
    bf = ml_dtypes.bfloat16
    # W1 [E, D, H] -> [t=(e,ht), d_in, (d_t, h_in)]: per t one contiguous
    # block whose SBUF layout is [128 d_in, 8 d_t * 128 h]
    w1p = np.ascontiguousarray(
        np.asarray(W1, f).reshape(E, DT, 128, HT, 128)
        .transpose(0, 3, 2, 1, 4).reshape(T, 128, DT * 128)).astype(bf)
    b1t = np.ascontiguousarray(
        np.asarray(b1, f).reshape(E, HT, 128).transpose(2, 0, 1).reshape(128, T))
    w2bd = np.zeros((128, T, E), dtype=f)
    for t in range(T):
        e, ht = divmod(t, HT)
        w2bd[:, t, e] = W2[e, ht * 128:(ht + 1) * 128]
    w2bd = w2bd.reshape(128, T * E).astype(bf)
    # gate weights replicated into the 4 col-groups (16 used + 16 pad cols)
    wgr = np.asarray(Wg, f).reshape(DT, 128, E)
    wg4 = np.zeros((DT, 128, 128), dtype=f)
    for j in range(GB):
        wg4[:, :, 32 * j:32 * j + E] = wgr
    wg4 = np.ascontiguousarray(wg4.transpose(1, 0, 2).reshape(128, DT * 128)).astype(bf)
    lane = np.arange(128) % 32
    real = lane < E
    bg4 = np.full((128, 1), -40.0, f)
    bg4[real, 0] = np.tile(np.asarray(bg, f), GB)
    b2q4 = np.zeros((128, 1), f)
    b2q4[real, 0] = np.tile(np.asarray(b2, f) / 4.0, GB)
    ones1 = np.where(real, 1.0, 0.0).astype(f).reshape(128, 1)
    o025 = np.where(real, 0.25, 0.0).astype(f).reshape(128, 1)
    shared = {"w1p": w1p, "b1t": b1t, "w2bd": w2bd, "wg4": wg4, "bg4": bg4,
              "b2q4": b2q4, "ones1": ones1, "o025": o025}
    xT = np.asarray(x, f).T.astype(bf)  # [D, B]
    xts = [np.ascontiguousarray(xT[:, c * BS:(c + 1) * BS]) for c in range(N_CORES)]
    return shared, xts


def run(inputs, trace=False):
    nc = build_bass()
    shared, xts = prep_inputs(**inputs)
    in_maps = [dict(shared, xt=xts[c]) for c in range(N_CORES)]
    res = run_bass_kernel_spmd(
        nc, in_maps, core_ids=list(range(N_CORES)), trace=trace
    )
    y = np.concatenate([r["y"] for r in res.results], axis=1)  # [1, B]
    return np.ascontiguousarray(y.reshape(B, 1).astype(np.float32)), res


def kernel(**inputs):
    y, _ = run(inputs, trace=False)
    return y


if __name__ == "__main__":
    rng = np.random.default_rng(0)
    ins = {
        "x": rng.standard_normal((B, D), dtype=np.float32),
        "W1": rng.standard_normal((E, D, H), dtype=np.float32) / 32,
        "b1": rng.standard_normal((E, H), dtype=np.float32) / 32,
        "W2": rng.standard_normal((E, H), dtype=np.float32) / 32,
        "b2": rng.standard_normal((E,), dtype=np.float32) / 32,
        "Wg": rng.standard_normal((D, E), dtype=np.float32) / 32,
        "bg": rng.standard_normal((E,), dtype=np.float32) / 32,
    }
    y = kernel(**ins)
    print("ok", y.shape, y.dtype)
